# revision 28
# baseline (speedup 1.0000x reference)
"""Trainium2 Bass kernel for nn_MAMLAwareGANLoss.

Reference computation (B=1024, Z=256, H=W=128, N=H*W=16384):
    fake   = tanh(noise @ Wg)                      # [B, N]
    d_fake = fake @ Wd                             # [B, 1]
    g_loss = mean(softplus(-d_fake))               # (+ 0.0 * sum(d_real) == 0)
    solvability_loss = mean(per-sample flood-fill penalty of (fake == 1.0) walls)
    cur    = mean(fake == 1.0)
    difficulty_loss  = (cur - current_difficulty)^2
    loss   = g_loss + w_s * solvability_loss + w_d * difficulty_loss

Key structural facts used here:
  * real_mazes enters only through `0.0 * sum(d_real)` == exactly 0.0 -> never loaded.
  * "walls" are cells where float32 tanh(x) rounds to exactly 1.0, which requires
    x >= ~9.01.  We prove on the host (Cauchy-Schwarz over the actual inputs:
    max_b ||noise_b|| * max_n ||Wg[:, n]||) that no |x| can exceed the threshold,
    hence wall count == 0 exactly => solvability_loss == 0.0 and cur == 0.0.
    If the bound ever fails we fall back to an exact host recomputation.
  * Therefore the device only computes d_fake = (tanh(noise @ Wg)) @ Wd.

Device sharding (8 cores): shard the N (=H*W) dimension, 2048 columns/core.
Each core computes, for all 1024 samples, the partial dot product
    dpart[b] = sum_{n in shard} tanh((noise @ Wg)[b, n]) * Wd[n]
The host sums the 8 partials, applies softplus and the scalar tail.

All device arithmetic is fp8e4 (e4m3).  Host-side scaling keeps every tensor in
the fp8 normal range: noise*16, Wg*64, Wd*64.  The activation instruction
applies tanh(psum / 1024), and the host divides the partial sums by 64.
Host-simulated end-to-end rel err of the fp8 pipeline vs the fp32 reference is
~8e-4 (tolerance 2e-2).

Per-core device program (layout: n on PSUM partitions, b on free axis):
  * main: x[n, b] = sum_z Wg[z, n] * noiseT[z, b] -- one DoubleRow fp8 matmul
    per (n-tile, b-half): both z k-tiles contracted in a single pass.
  * tanh: t[n, b] = tanh(x / 1024) on the ACT engine (PSUM -> SBUF fp8).
    This is the critical path: 2048 free elems/partition/tile-pair at
    ~1.2 GHz x 128 lanes ==> ~1 us per n-tile, ~16.2 us for the stream.
  * reduce: one DoubleRow matmul per (tile-pair, b-half) at PE origin with
    zero-padded per-pair Wd weights (M=32): row p of the PSUM accumulator
    receives pair p's partial sum, other rows accumulate zeros.  188 ns each,
    no PE column-group juggling, and the result sits in 8 contiguous
    partitions for a single narrow copy out.
"""

import numpy as np
import ml_dtypes

B, Z, H, W = 1024, 256, 128, 128
N = H * W               # 16384
NCORES = 8
NSH = N // NCORES       # 2048 columns of Wg per core
P = 128
NT = NSH // P           # 16 n-tiles per core
NB = B                  # 1024 samples (free axis)
BH = NB // 2            # b-half (one matmul / DMA granule)
NPAIR = NT // 2         # 8 tile pairs for the DoubleRow reduction

S_NOISE = 16.0          # host pre-scales (keep fp8 values in normal range)
S_WG = 64.0
S_WD = 64.0
X_SCALE = S_NOISE * S_WG

# float32 tanh(x) rounds to exactly 1.0 only for x >= ~9.01; stay well below.
WALL_SAFE_BOUND = 8.5

_PROG = None  # cached compiled Bass program


def _build_program():
    import concourse.bass as bass
    import concourse.tile as tile
    from concourse import bacc, mybir

    f32 = mybir.dt.float32
    f8 = mybir.dt.float8e4
    bf16 = mybir.dt.bfloat16
    Tanh = mybir.ActivationFunctionType.Tanh
    DR = mybir.MatmulPerfMode.DoubleRow

    nc = bacc.Bacc(
        "TRN2", target_bir_lowering=False, debug=False, num_devices=NCORES
    )
    # Host-relaid inputs, partition-major so every DMA is a straight
    # per-partition segment copy:
    #   noise_t : [128(p), 2(z-tile), 2(b-half), 512]   fp8, 256KB
    #   wg_shard: [128(p), 16(tile), 2(z-tile), 128]    fp8, 512KB
    #   wd_shard: [128(p), 8(pair), 2(k), 32]           fp8, 64KB zero-padded
    noise_d = nc.declare_dram_parameter("noise_t", [P, 2, 2, BH], f8, isOutput=False)
    wg_d = nc.declare_dram_parameter("wg_shard", [P, NT, 2, P], f8, isOutput=False)
    wd_d = nc.declare_dram_parameter("wd_shard", [P, NPAIR, 2, 32], f8, isOutput=False)
    # Row p holds pair p's partial dot products; the host sums the 8 rows
    # and divides by S_WD.
    out_d = nc.declare_dram_parameter("dpart", [8, NB], f32, isOutput=True)

    with tile.TileContext(nc) as tc:
        with (
            tc.tile_pool(name="const", bufs=1) as cpool,
            tc.tile_pool(name="ps", bufs=2, space="PSUM") as pspool,
            tc.tile_pool(name="psdve", bufs=1, space="PSUM") as dvepool,
            tc.tile_pool(name="dps", bufs=1, space="PSUM") as dpool,
        ):
            # Reduction accumulator: rows 0..7 <- pairs 0..7, free axis = b.
            dd = dpool.tile([32, NB], f32, tag="dd")

            noise_sb = cpool.tile([P, 2, 2, BH], f8, tag="noise")
            wg_sb = cpool.tile([P, NT, 2, P], f8, tag="wg")
            wd_sb = cpool.tile([P, NPAIR, 2, 32], f8, tag="wd")
            t_all = cpool.tile([P, NT, NB], f8, tag="t")

            # DMA plan: ONE queue (sync) in priority order.  Transfers on a
            # single ring execute strictly in issue order, so the gating
            # pieces (noise, first wg tiles) never compete with the bulk for
            # DMA-engine bandwidth — parallel queues round-robin the 16
            # engines and were observed to starve noise nondeterministically.
            nc.sync.dma_start(out=noise_sb[:], in_=noise_d[:])
            nc.gpsimd.dma_start(out=wg_sb[:, 0:2], in_=wg_d[:, 0:2])
            nc.gpsimd.dma_start(out=wg_sb[:, 2:8], in_=wg_d[:, 2:8])
            nc.gpsimd.dma_start(out=wg_sb[:, 8:16], in_=wg_d[:, 8:16])
            nc.scalar.dma_start(out=wd_sb[:], in_=wd_d[:])

            # --- PE warm-up: keep the tensor engine busy during the DMA wait
            # (HAM unthrottle + p-state ramp).  Output lands in dd row 0,
            # which the first real reduction clears via start=True.
            warm_sb = cpool.tile([P, 512], bf16, tag="warm")
            nc.vector.memset(warm_sb[:], 0.0)
            # Preload the tanh activation table (~1.3us) during the DMA wait.
            warm_act = cpool.tile([P, 16], f32, tag="warm_act")
            nc.scalar.activation(warm_act[:], warm_sb[:, 0:16], Tanh)
            # Bridge the PE from preamble end to the first gating DMA with NO
            # idle gap: HAM only grants the full-speed window ~5us after
            # CONTINUOUS tensor-engine activity begins, an idle gap resets
            # that clock, and the activity must look "hot" — M=1 warmups
            # (one PE column) were ignored by HAM.  Use the full 128x128
            # array; the scratch PSUM target aliases the first DVE tile's
            # buffer (WAR-ordered by the pool).
            dve_ps = dvepool.tile([P, NB], f32, tag="dve_ps")
            warm_ps = dve_ps
            for sz in (512, 512, 512, 512, 256):
                nc.tensor.matmul(
                    warm_ps[:, 0:sz],
                    warm_sb[:, 0:128],
                    warm_sb[:, 0:sz],
                    start=True,
                    stop=True,
                    skip_group_check=True,
                )

            def emit_reduce(p):
                # Pair p's weighted reduction over both b-halves.  All pairs
                # accumulate into dd[0:32]; the zero-padded weights route
                # pair p's result to row p.
                for h in range(2):
                    nc.tensor.matmul(
                        dd[0:32, h * BH : (h + 1) * BH],
                        wd_sb[:, p],
                        t_all[:, 2 * p : 2 * p + 2, h * BH : (h + 1) * BH],
                        start=(p == 0),
                        stop=(p == NPAIR - 1),
                        perf_mode=DR,
                        skip_group_check=True,
                    )

            # Tiles whose tanh runs on the (otherwise idle) DVE as a degree-5
            # odd minimax polynomial instead of the ACT engine.  ACT is the
            # critical path (~1.0us/tile); DVE takes ~4.2us/tile, so three
            # spread-out tiles run concurrently with the 13-tile ACT stream.
            # PSUM holds 1024*x, so the coefficients fold exact powers of two:
            #   tanh(x) ~ x*(c1 + c3 u + c5 u^2),  u = x^2
            #   t = ps*(c1' + c3' u' + c5' u'^2),  u' = ps^2, ps = 2^10 x
            # Max pointwise error 8.8e-3 on |x|<=1.9; end-to-end effect on the
            # loss is ~1e-3 (errors average out across the wd-weighted sum).
            DVE_TILES = (2, 6)
            C1, C3, C5 = 0.96252168, -0.21851052, 0.02564391
            C1P, C3P, C5P = C1 * 2.0**-10, C3 * 2.0**-30, C5 * 2.0**-50
            mult = mybir.AluOpType.mult
            add = mybir.AluOpType.add
            xb_sb = cpool.tile([P, NB], bf16, tag="xb")
            u_sb = cpool.tile([P, NB], bf16, tag="u")
            a1_sb = cpool.tile([P, NB], bf16, tag="a1")
            a2_sb = cpool.tile([P, NB], bf16, tag="a2")

            for i in range(NT):
                # DVE tiles hold their PSUM buffer ~4.7us (vs ACT's ~1.0us);
                # a dedicated buffer (shared with the warmups) keeps them out
                # of the main rotation so later tiles' matmuls never stall on
                # them.  Reusing the same tile object serializes its users
                # through the auto-inserted WAR dependencies.
                ps = dve_ps if i in DVE_TILES else pspool.tile([P, NB], f32)
                if i == 0:
                    # Interleave tile 0's matmuls and tanh halves so the ACT
                    # stream starts as soon as noise b-half0 lands.
                    for h in range(2):
                        nc.tensor.matmul(
                            ps[:, h * BH : (h + 1) * BH],
                            wg_sb[:, i],
                            noise_sb[:, :, h, :],
                            start=True,
                            stop=True,
                            perf_mode=DR,
                        )
                        nc.scalar.activation(
                            t_all[:, i, h * BH : (h + 1) * BH],
                            ps[:, h * BH : (h + 1) * BH],
                            Tanh,
                            scale=1.0 / X_SCALE,
                        )
                    continue
                for h in range(2):
                    nc.tensor.matmul(
                        ps[:, h * BH : (h + 1) * BH],
                        wg_sb[:, i],
                        noise_sb[:, :, h, :],
                        start=True,
                        stop=True,
                        perf_mode=DR,
                    )
                if i in DVE_TILES:
                    # xb = bf16(1024*x); the rest of the chain runs in bf16
                    # SBUF where DVE gets its 2x/4x element rates.
                    nc.vector.tensor_copy(xb_sb[:], ps[:])
                    nc.vector.tensor_tensor(
                        out=u_sb[:], in0=xb_sb[:], in1=xb_sb[:], op=mult
                    )
                    nc.vector.tensor_scalar(
                        out=a1_sb[:], in0=u_sb[:],
                        scalar1=C5P, scalar2=C3P, op0=mult, op1=add,
                    )
                    nc.vector.tensor_tensor(
                        out=a2_sb[:], in0=a1_sb[:], in1=u_sb[:], op=mult
                    )
                    nc.vector.tensor_scalar(
                        out=a1_sb[:], in0=a2_sb[:],
                        scalar1=1.0, scalar2=C1P, op0=mult, op1=add,
                    )
                    nc.vector.tensor_tensor(
                        out=t_all[:, i, :], in0=a1_sb[:], in1=xb_sb[:], op=mult
                    )
                elif i == NT - 1:
                    # Last tile: tanh per b-half so the final reduction and
                    # the output path start after half the tanh.
                    for h in range(2):
                        nc.scalar.activation(
                            t_all[:, i, h * BH : (h + 1) * BH],
                            ps[:, h * BH : (h + 1) * BH],
                            Tanh,
                            scale=1.0 / X_SCALE,
                        )
                else:
                    nc.scalar.activation(
                        t_all[:, i, :], ps[:], Tanh, scale=1.0 / X_SCALE
                    )
                # Reduce pairs lag the mains so the (in-order) PE never
                # stalls on a pending tanh; pairs containing a DVE tile lag
                # further (the DVE chain finishes late).
                sched = {5: 0, 9: 1, 11: 2, 13: 3}
                if i in sched:
                    emit_reduce(sched[i])
            for p in range(4, NPAIR - 1):
                emit_reduce(p)
            # Final pair + output path pipelined per b-half: the h0 copy and
            # its DMA overlap the h1 tanh/reduce.
            out_sb = cpool.tile([8, NB], f32, tag="out")
            for h in range(2):
                nc.tensor.matmul(
                    dd[0:32, h * BH : (h + 1) * BH],
                    wd_sb[:, NPAIR - 1],
                    t_all[:, NT - 2 : NT, h * BH : (h + 1) * BH],
                    start=False,
                    stop=True,
                    perf_mode=DR,
                    skip_group_check=True,
                )
                nc.vector.tensor_copy(
                    out_sb[:, h * BH : (h + 1) * BH], dd[0:8, h * BH : (h + 1) * BH]
                )
                nc.sync.dma_start(
                    out=out_d[:, h * BH : (h + 1) * BH],
                    in_=out_sb[:, h * BH : (h + 1) * BH],
                )

    nc.compile()
    return nc


def _get_program():
    global _PROG
    if _PROG is None:
        _PROG = _build_program()
    return _PROG


def _make_in_maps(noise, Wg, Wd):
    f8 = ml_dtypes.float8_e4m3fn
    # noise_t[p, zi, h, c] = fp8(noise[h*512+c, zi*128+p] * 16)
    nq = (noise * S_NOISE).astype(f8)
    noise_t = np.ascontiguousarray(
        nq.T.reshape(2, P, 2, BH).transpose(1, 0, 2, 3)
    )
    in_maps = []
    for c in range(NCORES):
        # wg_t[p, i, zi, cc] = fp8(Wg[zi*128+p, base + i*128+cc] * 64)
        wq = (Wg[:, c * NSH : (c + 1) * NSH] * S_WG).astype(f8)
        wg_t = np.ascontiguousarray(
            wq.reshape(2, P, NT, P).transpose(1, 2, 0, 3)
        )
        # wd_t[p, pair, k, col] = fp8(Wd[base + (2*pair+k)*128 + p] * 64)
        # on the diagonal col == pair, zero elsewhere.
        seg = (Wd[c * NSH : (c + 1) * NSH, 0] * S_WD).astype(f8)
        seg = seg.reshape(NPAIR, 2, P)  # [pair, k, p]
        wd_t = np.zeros((P, NPAIR, 2, 32), f8)
        for pr in range(NPAIR):
            wd_t[:, pr, 0, pr] = seg[pr, 0]
            wd_t[:, pr, 1, pr] = seg[pr, 1]
        in_maps.append({"noise_t": noise_t, "wg_shard": wg_t, "wd_shard": wd_t})
    return in_maps


def run_device(noise, Wg, Wd, trace=False):
    """Run the SPMD kernel on 8 cores; return (d_fake[B] float64, results)."""
    from concourse.bass_utils import run_bass_kernel_spmd

    nc = _get_program()
    in_maps = _make_in_maps(noise, Wg, Wd)
    res = run_bass_kernel_spmd(nc, in_maps, list(range(NCORES)), trace=trace)
    d_fake = np.zeros(NB, np.float64)
    for r in res.results:
        d_fake += np.asarray(r["dpart"], np.float64).reshape(8, NB).sum(axis=0)
    d_fake /= S_WD
    return d_fake, res


def _dilate(v):
    out = v.copy()
    out[:-1, :] |= v[1:, :]
    out[1:, :] |= v[:-1, :]
    out[:, :-1] |= v[:, 1:]
    out[:, 1:] |= v[:, :-1]
    return out


def _host_exact_maze_terms(noise, Wg):
    """Fallback (practically unreachable): exact wall/flood-fill computation."""
    solv = 0.0
    wall_total = 0
    for b0 in range(0, B, 64):
        x = noise[b0 : b0 + 64].astype(np.float32) @ Wg.astype(np.float32)
        fake = np.tanh(x).astype(np.float32)
        for j in range(fake.shape[0]):
            maze = fake[j].reshape(H, W)
            wall = maze == np.float32(1.0)
            nwall = int(wall.sum())
            wall_total += nwall
            pen = 0.0
            if float(wall.mean()) > 0.5:
                pen += 1.0
            if nwall >= 3:
                open_ = ~wall
                visited = np.zeros((H, W), bool)
                visited[1, 1] = True
                while True:
                    nv = visited | (_dilate(visited) & open_)
                    if not (nv & ~visited).any():
                        break
                    visited = nv
                wf = wall.astype(np.float32)
                wa = np.zeros((H, W), np.float32)
                wa[:-1, :] += wf[1:, :]
                wa[1:, :] += wf[:-1, :]
                wa[:, :-1] += wf[:, 1:]
                wa[:, 1:] += wf[:, :-1]
                pen += 0.1 * float((visited & (wa >= 3.0)).sum())
            solv += pen
    solv /= B
    cur = wall_total / float(B * H * W)
    return solv, cur


def kernel(**inputs) -> np.ndarray:
    noise = np.asarray(inputs["noise"], np.float32)
    Wg = np.asarray(inputs["Wg"], np.float32)
    Wd = np.asarray(inputs["Wd"], np.float32)
    p = float(np.asarray(inputs["maml_performance"]).reshape(-1)[0])
    cd = float(np.asarray(inputs["current_difficulty"]).reshape(-1)[0])

    d_fake, _ = run_device(noise, Wg, Wd)

    # g_loss = mean(softplus(-d_fake));  0.0 * sum(d_real) == 0 exactly.
    g_loss = float(np.mean(np.logaddexp(0.0, -d_fake)))

    # Wall existence bound: |x[b,n]| <= max_b||noise_b|| * max_n||Wg[:,n]||.
    rn = float(np.sqrt((noise.astype(np.float64) ** 2).sum(axis=1)).max())
    cn = float(np.sqrt((Wg.astype(np.float64) ** 2).sum(axis=0)).max())
    if rn * cn * 1.0001 < WALL_SAFE_BOUND:
        solv, cur = 0.0, 0.0
    else:  # pragma: no cover - requires |pre-tanh| ~ 28 sigma
        solv, cur = _host_exact_maze_terms(noise, Wg)

    w_s = 0.8 if p < 0.4 else (0.4 if p > 0.6 else 0.6)
    w_d = 0.05 if p < 0.4 else (0.2 if p > 0.6 else 0.1)
    difficulty = (cur - cd) ** 2
    loss = g_loss + w_s * solv + w_d * difficulty
    return np.array(loss, dtype=np.float32)
